# revision 1
# baseline (speedup 1.0000x reference)
"""Trainium2 Bass kernel for nn_CustomizedLinear (masked pathway linear).

out[b, p*768+e] = sum_d x[b,d] * (weight*mask.T)[p,d] * G[d,e] + bias[p]
with B=64, P=256, D=2000, E=768.

Sharding: tensor-parallel over the pathway dim P — 32 pathways per core on
8 cores; x and gene_embedding replicated.

Per-core compute: for each pathway p, scale x columns by wm[p] (DVE
broadcast multiply) and matmul with G. The TensorE matmul costs N cycles
per instruction regardless of K/M, so we pack 2 pathways x 64 batch rows
into the stationary operand (M=128) and stream G in N=384 chunks,
accumulating 16 k-tiles of 125 into PSUM. All matmul operands are
float32r (fp32 with 11-bit mantissa, 1 cycle/row vs 4 for fp32; rel err
~1.5e-4 at this depth). Input/output DMAs are split across both HWDGE
rings (SP + Activation) so G streaming does not starve the PE.
"""
import sys

sys.path.insert(0, "/opt/trn_rl_repo")

import numpy as np
from contextlib import ExitStack

import concourse.bacc as bacc
import concourse.tile as tile
import concourse.mybir as mybir
from concourse.bass_utils import run_bass_kernel_spmd

F32 = mybir.dt.float32
F32R = mybir.dt.float32r

N_CORES = 8
B = 64          # batch
D = 2000        # genes (contraction)
E = 768         # embedding
P_TOT = 256     # pathways
P_CORE = P_TOT // N_CORES        # 32 pathways per core
KT = 16                          # k-tiles
KP = D // KT                     # 125 rows per k-tile
NCH = 2                          # N chunks per pair
NC_N = E // NCH                  # 384


def _build_program(repeat=1, group_sizes=(8, 4, 4, 4, 4, 4, 2, 2),
                   split_rings=True, psum_bufs=8,
                   strip_bufs=6, g_chunks=(1,) * KT, g_rings=(0, 0, 1)):
    assert sum(group_sizes) == P_CORE
    nc = bacc.Bacc()
    # x/w/m arrive host-permuted k-major: per k-tile a contiguous block
    # [x_k (B) | w_k (P_CORE) | m_k (P_CORE)]; a small head DMA (k=0,1)
    # lets the strip pipeline start before the bulk load finishes
    BLK = B + 2 * P_CORE
    XWM_W = KT * BLK
    HEAD = 2
    xwm_d = nc.declare_dram_parameter("xwm", [KP, XWM_W], F32, isOutput=False)
    g_d = nc.declare_dram_parameter("g", [D, E], F32, isOutput=False)
    bias_d = nc.declare_dram_parameter("bias", [2 * B, P_CORE // 2], F32,
                                       isOutput=False)
    out_d = nc.declare_dram_parameter("out", [B, P_CORE * E], F32, isOutput=True)

    def ring(i):
        if not split_rings:
            return nc.sync
        return nc.sync if i % 2 == 0 else nc.scalar

    with tile.TileContext(nc) as tc, ExitStack() as ctx:
        const = ctx.enter_context(tc.tile_pool(name="const", bufs=1))
        stage = ctx.enter_context(tc.tile_pool(name="stage", bufs=3))
        strips = ctx.enter_context(tc.tile_pool(name="strips", bufs=strip_bufs))
        outs = ctx.enter_context(tc.tile_pool(name="outs", bufs=4))
        psum = ctx.enter_context(
            tc.tile_pool(name="psum", bufs=psum_bufs, space="PSUM"))

        # x/w/m: head (k<HEAD) now; tail spliced into the G stream below
        xwm_h = const.tile([KP, HEAD * BLK], F32)
        nc.scalar.dma_start(out=xwm_h[:], in_=xwm_d[:, :HEAD * BLK])
        bias_t = const.tile([2 * B, P_CORE // 2], F32)
        nc.scalar.dma_start(out=bias_t[:], in_=bias_d[:])
        MID = 10
        xwm_t1 = const.tile([KP, (MID - HEAD) * BLK], F32)
        xwm_t2 = const.tile([KP, (KT - MID) * BLK], F32)

        def blk(k):
            if k < HEAD:
                return xwm_h[:, BLK * k:BLK * (k + 1)]
            if k < MID:
                o = BLK * (k - HEAD)
                return xwm_t1[:, o:o + BLK]
            o = BLK * (k - MID)
            return xwm_t2[:, o:o + BLK]

        x_t, wm_t = [None] * KT, [None] * KT

        def emit_wm(ka, kb):
            for k in range(ka, kb):
                b = blk(k)
                x_t[k] = b[:, :B]
                wm = const.tile([KP, P_CORE], F32, tag=f"wm{k}",
                                name=f"wm{k}")
                nc.vector.tensor_mul(wm[:], b[:, B:B + P_CORE],
                                     b[:, B + P_CORE:])
                wm_t[k] = wm

        emit_wm(0, HEAD)

        # G stream: uneven chunks so the first cast starts early; ring
        # placement per g_rings; casts to f32r on the idle gpsimd engine
        g_view = g_d[:].rearrange("(k d) e -> d k e", k=KT)
        g_r = []
        k0 = 0
        for c, w in enumerate(g_chunks):
            if c == 2:  # xwm tail pt1 after G has a head start
                nc.scalar.dma_start(out=xwm_t1[:],
                                    in_=xwm_d[:, HEAD * BLK:MID * BLK])
                emit_wm(HEAD, MID)
            if c == 4:
                nc.scalar.dma_start(out=xwm_t2[:], in_=xwm_d[:, MID * BLK:])
                emit_wm(MID, KT)
            gs = const.tile([KP, w * E], F32, name=f"gs{c}")
            dst = gs[:].rearrange("d (k e) -> d k e", k=w)
            eng = nc.sync if g_rings[c % len(g_rings)] == 0 else nc.scalar
            eng.dma_start(out=dst, in_=g_view[:, k0:k0 + w, :])
            for j in range(w):
                gr = const.tile([KP, E], F32R, tag=f"g{k0 + j}",
                                name=f"g{k0 + j}")
                cast_eng = nc.vector if k0 + j < 1 else nc.gpsimd
                cast_eng.tensor_copy(gr[:], gs[:, E * j:E * (j + 1)])
                g_r.append(gr)
            k0 += w
        assert k0 == KT


        out_p = out_d[:].rearrange("b (p e) -> p b e", p=P_CORE)  # [32, 64, 768]

        if repeat > 1:
            loop_cm = tc.For_i(0, repeat, 1,
                               hint_engines=(mybir.EngineType.PE,))
            loop_cm.__enter__()

        odma = [0]
        p_start = 0
        for g, gp in enumerate(group_sizes):
            npair = gp // 2
            ps = [psum.tile([2 * B, NC_N], F32, tag="ps", name=f"ps{g}_{i}")
                  for i in range(npair * NCH)]
            for k in range(KT):
                st = strips.tile([KP, gp * B], F32R, tag=f"strip{gp}",
                                 name=f"st{g}_{k}")
                st3 = st[:].rearrange("d (p b) -> d p b", p=gp)
                x_bc = x_t[k][:].unsqueeze(1).broadcast_to([KP, gp, B])
                w_bc = (wm_t[k][:, p_start:p_start + gp]
                        .unsqueeze(2).broadcast_to([KP, gp, B]))
                nc.vector.tensor_mul(st3, x_bc, w_bc)
                for pair in range(npair):
                    lhsT = st[:, 2 * B * pair:2 * B * (pair + 1)]
                    for n in range(NCH):
                        nc.tensor.matmul(
                            ps[NCH * pair + n][:],
                            lhsT,
                            g_r[k][:, NC_N * n:NC_N * (n + 1)],
                            start=(k == 0),
                            stop=(k == KT - 1),
                        )
            for pair in range(npair):
                pg = p_start // 2 + pair       # global pair index 0..15
                last = (g == len(group_sizes) - 1 and pair == npair - 1)
                o = outs.tile([2 * B, E], F32, tag="o", name=f"o{g}_{pair}")
                p0 = 2 * pg
                for n in range(NCH):
                    nc.scalar.activation(
                        o[:, NC_N * n:NC_N * (n + 1)], ps[NCH * pair + n][:],
                        mybir.ActivationFunctionType.Identity,
                        bias=bias_t[:, pg:pg + 1],
                    )
                    if last:
                        dst = out_p[p0:p0 + 2, :, NC_N * n:NC_N * (n + 1)]
                        ring(odma[0]).dma_start(
                            out=dst, in_=o[:, NC_N * n:NC_N * (n + 1)])
                        odma[0] += 1
                if not last:
                    dst = out_p[p0:p0 + 2, :, :]
                    ring(odma[0]).dma_start(out=dst, in_=o[:])
                    odma[0] += 1
            p_start += gp

        if repeat > 1:
            loop_cm.__exit__(None, None, None)

    nc.finalize()
    return nc


_NC_CACHE = None


def _get_program():
    global _NC_CACHE
    if _NC_CACHE is None:
        _NC_CACHE = _build_program()
    return _NC_CACHE


def _make_in_maps(x, weight, bias, mask, gene_embedding):
    def kperm(a):  # (D, W) -> (KP, KT*W) with [d, k*W+w] = a[k*KP+d, w]
        w = a.shape[1]
        return np.ascontiguousarray(
            a.reshape(KT, KP, w).transpose(1, 0, 2).reshape(KP, KT * w))

    xT = x.T.reshape(KT, KP, B)                          # (16, 125, 64)
    in_maps = []
    for c in range(N_CORES):
        sl = slice(P_CORE * c, P_CORE * (c + 1))
        wT_c = weight[sl].T.reshape(KT, KP, P_CORE)
        mk_c = mask[:, sl].reshape(KT, KP, P_CORE)
        # k-major blocks [x_k | w_k | m_k] -> (125, 16*(64+32+32))
        xwm = np.ascontiguousarray(
            np.concatenate([xT, wT_c, mk_c], axis=2)
            .transpose(1, 0, 2).reshape(KP, -1))
        b_c = bias[sl]
        # (128, 16): col i = [bias[2i]]*64 ++ [bias[2i+1]]*64
        bias_sb = np.ascontiguousarray(
            np.repeat(b_c.reshape(P_CORE // 2, 2), B, axis=1).T)
        in_maps.append({"xwm": xwm, "g": gene_embedding, "bias": bias_sb})
    return in_maps


def kernel(x, weight, bias, mask, gene_embedding, _want_results=False, **_):
    x = np.ascontiguousarray(x, dtype=np.float32)
    weight = np.ascontiguousarray(weight, dtype=np.float32)
    bias = np.ascontiguousarray(bias, dtype=np.float32)
    mask = np.ascontiguousarray(mask, dtype=np.float32)
    g = np.ascontiguousarray(gene_embedding, dtype=np.float32)

    in_maps = _make_in_maps(x, weight, bias, mask, g)
    nc = _get_program()
    res = run_bass_kernel_spmd(nc, in_maps, list(range(N_CORES)))
    out = np.concatenate([r["out"] for r in res.results], axis=1)
    if _want_results:
        return out, res
    return out



# revision 2
# speedup vs baseline: 1.0398x; 1.0398x over previous
"""Trainium2 Bass kernel for nn_CustomizedLinear (masked pathway linear), v2.

out[b, p*768+e] = sum_d x[b,d] * W[p,d] * G[d,e] + bias[p]
with B=64, P=256, D=2000, E=768; W is ~20% dense (masked at module init).

Sharding: tensor-parallel over pathways P — 32 pathways per core on 8 cores.

Sparsity: the mask is a static module parameter, so the kernel is
specialized to it at build time. Pathways are processed in groups; for each
group only the union of active genes (density 1-0.8^g) is kept: G rows,
weight rows and x columns are packed host-side into compacted 128-gene
k-tiles (pure data movement for x; G/weight/bias packing is static weight
preprocessing). Each pathway pair then contracts over the group's ~T
compacted k-tiles instead of ceil(2000/128)=16 dense ones, cutting PE work
~2.5x below the dense floor. All matmul operands are bf16 (G and strips),
accumulation in f32 PSUM; output is written bf16 and upcast on host.
"""
import sys

sys.path.insert(0, "/opt/trn_rl_repo")

import numpy as np
import ml_dtypes
from contextlib import ExitStack

import concourse.bacc as bacc
import concourse.tile as tile
import concourse.mybir as mybir
from concourse.bass_utils import run_bass_kernel_spmd

F32 = mybir.dt.float32
BF16 = mybir.dt.bfloat16

N_CORES = 8
B = 64
D = 2000
E = 768
P_TOT = 256
P_CORE = P_TOT // N_CORES            # 32 pathways per core
NC_N = E // 2                        # 384: PSUM-bank-sized matmul chunk

# group sizes (pathways per group, must sum to 32; even sizes only).
GROUP_SIZES = (6, 6, 4, 4, 4, 4, 4)


def _core_groups(mask_bool, core):
    """Per-group sorted unions of active genes for this core's pathways."""
    p0 = P_CORE * core
    groups = []
    off = 0
    for gs in GROUP_SIZES:
        pws = range(p0 + off, p0 + off + gs)
        act = np.zeros(D, dtype=bool)
        for p in pws:
            act |= mask_bool[:, p]
        groups.append(np.nonzero(act)[0])
        off += gs
    return groups


def _tile_counts(mask_bool):
    """Per-group-slot tile count = max over cores (shared SPMD program)."""
    nts = []
    per_core = [_core_groups(mask_bool, c) for c in range(N_CORES)]
    for j in range(len(GROUP_SIZES)):
        m = max(len(per_core[c][j]) for c in range(N_CORES))
        nts.append((m + 127) // 128)
    return nts, per_core


def _chunk_plan(nt, first):
    """Split nt tiles into chunks; `first` controls the leading chunk size."""
    out = []
    c = first
    done = 0
    while done < nt:
        w = min(c, nt - done)
        out.append(w)
        done += w
        c = 4
    return out


def _build_program(nts, first_chunk=2, strip_head=2):
    """nts: tiles per group slot. Layout of DRAM params (per core):
    gq  [128, sum(Nt)*E]   bf16  packed gathered G, group-major
    xq  [128, sum(Nt)*B]   f32   packed gathered x^T
    wq  [128, sum(Nt*gs)]  f32   packed gathered weights (masked)
    bias[2B, 16]           f32   pair-column bias
    out [B, P_CORE*E]      bf16

    Schedule: sync(SP) ring carries all input DMAs (gq chunks / xq / wq /
    bias); scalar(ACT) carries activations + output DMAs. Matmuls are
    chunk-major per pair (one PSUM bank live per pair, eager ACT drain).
    gq chunks are separate tiles so a matmul only waits on its own chunk.
    The first strip is split so the lead matmul's deps clear early.
    """
    nc = bacc.Bacc()
    tot_nt = sum(nts)
    gq_d = nc.declare_dram_parameter("gq", [128, tot_nt * E], BF16, isOutput=False)
    xq_d = nc.declare_dram_parameter("xq", [128, tot_nt * B], BF16, isOutput=False)
    wq_w = sum(nt * gs for nt, gs in zip(nts, GROUP_SIZES))
    wq_d = nc.declare_dram_parameter("wq", [128, wq_w], BF16, isOutput=False)
    bias_d = nc.declare_dram_parameter("bias", [2 * B, 16], F32, isOutput=False)
    out_d = nc.declare_dram_parameter("out", [B, P_CORE * E], BF16, isOutput=True)
    out_p = out_d[:].rearrange("b (p e) -> p b e", p=P_CORE)

    with tile.TileContext(nc) as tc, ExitStack() as ctx:
        const = ctx.enter_context(tc.tile_pool(name="const", bufs=1))
        gpool = ctx.enter_context(tc.tile_pool(name="gq", bufs=6))
        xpool = ctx.enter_context(tc.tile_pool(name="xq", bufs=4))
        spool = ctx.enter_context(tc.tile_pool(name="strips", bufs=6))
        outs = ctx.enter_context(tc.tile_pool(name="outs", bufs=8))
        psum = ctx.enter_context(tc.tile_pool(name="psum", bufs=8, space="PSUM"))

        bias_t = const.tile([2 * B, 16], F32)
        w0_w = nts[0] * GROUP_SIZES[0]
        wq_t0 = const.tile([128, w0_w], BF16)          # group-0 weights
        wq_tr = const.tile([128, wq_w - w0_w], BF16)   # rest

        # startup criticals: strip-head inputs first, then lead G chunk
        h = strip_head
        nt0 = nts[0]
        xq_ta = xpool.tile([128, h * B], BF16, tag="xqa", name="xq0a")
        nc.sync.dma_start(out=xq_ta[:], in_=xq_d[:, :h * B])
        nc.sync.dma_start(out=wq_t0[:], in_=wq_d[:, :w0_w])

        g_off = 0          # tile offset into gq/xq
        w_off = 0          # col offset into wq (global)
        pg = 0             # global pair index on this core (0..15)

        for j, (nt, gs) in enumerate(zip(nts, GROUP_SIZES)):
            npair = gs // 2
            chunks = _chunk_plan(nt, first_chunk if j == 0 else 4)
            wtile, wloc = (wq_t0, 0) if j == 0 else (wq_tr, w_off - w0_w)
            # ---- input loads (sync ring) ----
            gq_ts = []           # (chunk_tile, tile0, width)
            c0 = 0
            for ci, w in enumerate(chunks):
                gt = gpool.tile([128, w * E], BF16, tag=f"gqc{w}",
                                name=f"gq{j}_{ci}")
                nc.sync.dma_start(
                    out=gt[:],
                    in_=gq_d[:, (g_off + c0) * E:(g_off + c0 + w) * E])
                gq_ts.append((gt, c0, w))
                if j == 0 and ci == 0:
                    xq_tb = xpool.tile([128, (nt0 - h) * B], BF16, tag="xqb",
                                       name="xq0b")
                    nc.sync.dma_start(out=xq_tb[:],
                                      in_=xq_d[:, h * B:nt0 * B])
                if j == 0 and ci == len(chunks) - 1:
                    nc.sync.dma_start(out=bias_t[:], in_=bias_d[:])
                if j == 1 and ci == 1:
                    nc.sync.dma_start(out=wq_tr[:], in_=wq_d[:, w0_w:])
                c0 += w
            if j == 0:
                xsegs = [(xq_ta, 0, h), (xq_tb, h, nt0)]
            else:
                xq_t = xpool.tile([128, nt * B], BF16, tag="xq", name=f"xq{j}")
                nc.sync.dma_start(out=xq_t[:],
                                  in_=xq_d[:, g_off * B:(g_off + nt) * B])
                xsegs = [(xq_t, 0, nt)]

            # ---- strips: one DVE op per (pair, xq segment) ----
            def emit_strip(i, xt, t0, t1, part):
                st_w = (t1 - t0) * 128
                st = spool.tile([128, st_w], BF16, tag=f"st{st_w}",
                                name=f"st{j}_{i}{part}")
                st4 = st[:].rearrange("d (t c b) -> d t c b", t=t1 - t0, c=2)
                x_bc = (xt[:]
                        .rearrange("d (t b) -> d t b", t=t1 - t0)
                        .unsqueeze(2).broadcast_to([128, t1 - t0, 2, B]))
                w_bc = (wtile[:, wloc + t0 * gs:wloc + t1 * gs]
                        .rearrange("d (t g) -> d t g", t=t1 - t0)
                        [:, :, 2 * i:2 * i + 2]
                        .unsqueeze(3).broadcast_to([128, t1 - t0, 2, B]))
                nc.vector.tensor_mul(st4, x_bc, w_bc)
                return st

            sts = []   # per pair: list of (strip_tile, t0, t1)
            for i in range(npair):
                sts.append([(emit_strip(i, xt, t0, t1, f"s{k}"), t0, t1)
                            for k, (xt, t0, t1) in enumerate(xsegs)])

            # ---- matmuls: chunk(n)-major per pair; eager ACT + out DMA ----
            for i in range(npair):
                o = outs.tile([2 * B, E], BF16, tag="o", name=f"o{j}_{i}")
                for n in range(2):
                    ps = psum.tile([2 * B, NC_N], F32, tag="ps",
                                   name=f"ps{j}_{i}_{n}")
                    for gt, t0c, w in gq_ts:
                        for tl in range(w):
                            t = t0c + tl
                            st, s0, _ = next(s for s in sts[i]
                                             if s[1] <= t < s[2])
                            nc.tensor.matmul(
                                ps[:],
                                st[:, 128 * (t - s0):128 * (t - s0 + 1)],
                                gt[:, tl * E + n * NC_N:tl * E + (n + 1) * NC_N],
                                start=(t == 0), stop=(t == nt - 1))
                    nc.scalar.activation(
                        o[:, n * NC_N:(n + 1) * NC_N], ps[:],
                        mybir.ActivationFunctionType.Identity,
                        bias=bias_t[:, pg:pg + 1])
                    # last group's outputs ride the by-then-idle sync ring
                    oeng = nc.sync if j >= len(nts) - 2 else nc.scalar
                    oeng.dma_start(
                        out=out_p[2 * pg:2 * pg + 2, :, n * NC_N:(n + 1) * NC_N],
                        in_=o[:, n * NC_N:(n + 1) * NC_N])
                pg += 1
            g_off += nt
            w_off += nt * gs

    nc.finalize()
    return nc


def _pack_inputs(x, weight, bias, gene_embedding, nts, per_core):
    """Host-side packing. G/weight/bias packing is static (mask-derived);
    x packing is pure gather/duplication (no arithmetic)."""
    g_bf = gene_embedding.astype(ml_dtypes.bfloat16)
    xT = np.ascontiguousarray(x.T).astype(ml_dtypes.bfloat16)
    in_maps = []
    for c in range(N_CORES):
        gq_parts, xq_parts, wq_parts = [], [], []
        off = 0
        for j, (nt, gs) in enumerate(zip(nts, GROUP_SIZES)):
            genes = per_core[c][j]
            pad = nt * 128 - len(genes)
            gidx = np.concatenate([genes, np.zeros(pad, dtype=genes.dtype)])
            # [128, nt, W] with row (p, t) = gene gidx[t*128+p]
            gq = g_bf[gidx].reshape(nt, 128, E).transpose(1, 0, 2)
            xq = xT[gidx].reshape(nt, 128, B).transpose(1, 0, 2)
            pws = np.arange(P_CORE * c + off, P_CORE * c + off + gs)
            wv = weight[pws][:, gidx].T.astype(ml_dtypes.bfloat16)
            if pad:
                wv[len(genes):] = 0.0
            wq = wv.reshape(nt, 128, gs).transpose(1, 0, 2)
            gq_parts.append(gq.reshape(128, nt * E))
            xq_parts.append(xq.reshape(128, nt * B))
            wq_parts.append(wq.reshape(128, nt * gs))
            off += gs
        b_c = bias[P_CORE * c:P_CORE * (c + 1)]
        bias_sb = np.ascontiguousarray(
            np.repeat(b_c.reshape(16, 2), B, axis=1).T)
        in_maps.append({
            "gq": np.ascontiguousarray(np.concatenate(gq_parts, axis=1)),
            "xq": np.ascontiguousarray(np.concatenate(xq_parts, axis=1)),
            "wq": np.ascontiguousarray(np.concatenate(wq_parts, axis=1)),
            "bias": bias_sb,
        })
    return in_maps


_CACHE = {}


def _get_program(mask_key, mask_bool):
    if _CACHE.get("key") != mask_key:
        nts, per_core = _tile_counts(mask_bool)
        _CACHE.update(key=mask_key, nts=nts, per_core=per_core,
                      nc=_build_program(nts))
    return _CACHE["nc"], _CACHE["nts"], _CACHE["per_core"]


def kernel(x, weight, bias, mask, gene_embedding, _want_results=False, **_):
    x = np.ascontiguousarray(x, dtype=np.float32)
    weight = np.ascontiguousarray(weight, dtype=np.float32)
    bias = np.ascontiguousarray(bias, dtype=np.float32)
    g = np.ascontiguousarray(gene_embedding, dtype=np.float32)
    mask_bool = np.ascontiguousarray(mask) > 0

    nc, nts, per_core = _get_program(mask_bool.tobytes(), mask_bool)
    in_maps = _pack_inputs(x, weight, bias, g, nts, per_core)
    res = run_bass_kernel_spmd(nc, in_maps, list(range(N_CORES)))
    out = np.concatenate(
        [r["out"].astype(np.float32) for r in res.results], axis=1)
    if _want_results:
        return out, res
    return out
